# revision 1
# baseline (speedup 1.0000x reference)
"""GQA multi-head attention (B=2, S=2048, D=2048, HQ=16, HKV=4, DK=128) with
RoPE + causal softmax + output projection, sharded over 8 NeuronCores as
(batch x kv-head-group): core c handles batch c//4, kv head c%4 (4 query
heads). w_q/w_kv column-sharded, fc row-sharded; partial fc outputs are
summed on the host (the "all-reduce").
"""

import sys

for _p in ("/opt/trn_rl_repo", "/root/.axon_site", "/root/.axon_site/_ro/trn_rl_repo"):
    if _p not in sys.path:
        sys.path.insert(0, _p)

import numpy as np

import concourse.bass as bass
import concourse.mybir as mybir
import concourse.tile as tile
from concourse import bacc
from concourse.bass_utils import run_bass_kernel_spmd

F32 = mybir.dt.float32
F16 = mybir.dt.float16

B, S, D = 2, 2048, 2048
HKV, NREP, DK = 4, 4, 128
HG = NREP  # query heads per core
KC = D // 128  # contraction chunks
SQC = S // 512  # 512-wide query column chunks
SCALE = float(1.0 / np.sqrt(DK))

_COMPILED = None


def _build():
    nc = bacc.Bacc(None, target_bir_lowering=False, debug=False)

    xT = nc.dram_tensor("xT", [D, S], F16, kind="ExternalInput")
    wq = nc.dram_tensor("wq", [D, HG * DK], F16, kind="ExternalInput")
    wk = nc.dram_tensor("wk", [D, DK], F16, kind="ExternalInput")
    wv = nc.dram_tensor("wv", [D, DK], F16, kind="ExternalInput")
    fcw = nc.dram_tensor("fcw", [HG * DK, D], F16, kind="ExternalInput")
    cosT = nc.dram_tensor("cosT", [64, S], F32, kind="ExternalInput")
    sinT = nc.dram_tensor("sinT", [64, S], F32, kind="ExternalInput")
    masks = nc.dram_tensor("masks", [128, 4, 512], F16, kind="ExternalInput")
    onesc = nc.dram_tensor("onesc", [128, 1], F16, kind="ExternalInput")
    out = nc.dram_tensor("out", [S, D], F32, kind="ExternalOutput")

    with tile.TileContext(nc) as tc:
        with tc.tile_pool(name="persist", bufs=1) as persist:
            # attention-phase residents
            qt_sb = persist.tile([128, HG, S], F16)  # Q^T, rope'd, per head
            kt_sb = persist.tile([128, S], F16)  # K^T rope'd
            v_sb = persist.tile([128, KC, DK], F16)  # V  [sk, dk] chunks
            ctxT = persist.tile([128, HG, S], F16)  # (softmax @ V)^T per head
            cos_sb = persist.tile([64, S], F32)
            sin_sb = persist.tile([64, S], F32)
            mask_sb = persist.tile([128, 4, 512], F16)
            ones_sb = persist.tile([128, 1], F16)
            nc.sync.dma_start(out=cos_sb, in_=cosT[:])
            nc.sync.dma_start(out=sin_sb, in_=sinT[:])
            nc.sync.dma_start(out=mask_sb, in_=masks[:])
            nc.sync.dma_start(out=ones_sb, in_=onesc[:])

            # ---------------- phase 1: projections + rope ----------------
            with tc.tile_pool(name="p1sb", bufs=1) as p1sb, \
                 tc.tile_pool(name="p1tmp", bufs=3) as p1tmp, \
                 tc.tile_pool(name="ps_proj", bufs=2, space="PSUM") as ps_proj, \
                 tc.tile_pool(name="ps_v", bufs=2, space="PSUM") as ps_v:
                xt_sb = p1sb.tile([128, KC, S], F16)
                wq_sb = p1sb.tile([128, KC, HG * DK], F16)
                wk_sb = p1sb.tile([128, KC, DK], F16)
                wv_sb = p1sb.tile([128, KC, DK], F16)
                nc.sync.dma_start(out=xt_sb, in_=xT.rearrange("(k p) s -> p k s", p=128))
                nc.sync.dma_start(out=wq_sb, in_=wq.rearrange("(k p) m -> p k m", p=128))
                nc.sync.dma_start(out=wk_sb, in_=wk.rearrange("(k p) m -> p k m", p=128))
                nc.sync.dma_start(out=wv_sb, in_=wv.rearrange("(k p) m -> p k m", p=128))

                def rope(dst, ps, qc):
                    c = cos_sb[:, qc * 512:(qc + 1) * 512]
                    s = sin_sb[:, qc * 512:(qc + 1) * 512]
                    pe, po = ps[0:64, :], ps[64:128, :]
                    t1 = p1tmp.tile([64, 512], F32, name="t1", tag="t1")
                    t2 = p1tmp.tile([64, 512], F32, name="t2", tag="t2")
                    nc.vector.tensor_tensor(t1, pe, c, op=mybir.AluOpType.mult)
                    nc.vector.tensor_tensor(t2, po, s, op=mybir.AluOpType.mult)
                    nc.vector.tensor_tensor(dst[0:64, :], t1, t2, op=mybir.AluOpType.subtract)
                    t3 = p1tmp.tile([64, 512], F32, name="t3", tag="t3")
                    t4 = p1tmp.tile([64, 512], F32, name="t4", tag="t4")
                    nc.vector.tensor_tensor(t3, pe, s, op=mybir.AluOpType.mult)
                    nc.vector.tensor_tensor(t4, po, c, op=mybir.AluOpType.mult)
                    nc.vector.tensor_tensor(dst[64:128, :], t3, t4, op=mybir.AluOpType.add)

                # K^T = wk^T @ xT  (then rope)
                for qc in range(SQC):
                    psk = ps_proj.tile([128, 512], F32, name="psk", tag="pp")
                    for k in range(KC):
                        nc.tensor.matmul(psk, wk_sb[:, k, :],
                                         xt_sb[:, k, qc * 512:(qc + 1) * 512],
                                         start=(k == 0), stop=(k == KC - 1))
                    rope(kt_sb[:, qc * 512:(qc + 1) * 512], psk, qc)

                # V = xT^T @ wv : out [sk, dk], 4 tiles packed per psum bank
                for gq in range(4):
                    psv = ps_v.tile([128, 512], F32, name="psv", tag="pv")
                    for vt in range(4):
                        skt = gq * 4 + vt
                        for k in range(KC):
                            nc.tensor.matmul(psv[:, vt * 128:(vt + 1) * 128],
                                             xt_sb[:, k, skt * 128:(skt + 1) * 128],
                                             wv_sb[:, k, :],
                                             start=(k == 0), stop=(k == KC - 1))
                    nc.vector.tensor_copy(
                        v_sb[:, gq * 4:(gq + 1) * 4, :].rearrange("p a b -> p (a b)"),
                        psv)

                # Q^T = wq^T @ xT (then rope)
                for mh in range(HG):
                    for qc in range(SQC):
                        psq = ps_proj.tile([128, 512], F32, name="psq", tag="pp")
                        for k in range(KC):
                            nc.tensor.matmul(psq, wq_sb[:, k, mh * 128:(mh + 1) * 128],
                                             xt_sb[:, k, qc * 512:(qc + 1) * 512],
                                             start=(k == 0), stop=(k == KC - 1))
                        rope(qt_sb[:, mh, qc * 512:(qc + 1) * 512], psq, qc)

            # ---------------- phase 2: attention ----------------
            with tc.tile_pool(name="es_pool", bufs=6) as es_pool, \
                 tc.tile_pool(name="nrm_pool", bufs=2) as nrm_pool, \
                 tc.tile_pool(name="ps_s", bufs=3, space="PSUM") as ps_s, \
                 tc.tile_pool(name="ps_ctx", bufs=2, space="PSUM") as ps_ctx, \
                 tc.tile_pool(name="ps_den", bufs=2, space="PSUM") as ps_den:
                for h in range(HG):
                    for qc in range(SQC):
                        nkc = 4 * (qc + 1)  # causal: sk chunks 0..nkc-1
                        psc = ps_ctx.tile([128, 512], F32, name="psc", tag="psc")
                        psd = ps_den.tile([1, 512], F32, name="psd", tag="psd")
                        qs = qt_sb[:, h, qc * 512:(qc + 1) * 512]
                        es_tiles = [None] * nkc

                        def scores(kc):
                            pss = ps_s.tile([128, 512], F32, name="pss", tag="pss")
                            nc.tensor.matmul(pss, kt_sb[:, kc * 128:(kc + 1) * 128],
                                             qs, start=True, stop=True)
                            es = es_pool.tile([128, 512], F16, name="es", tag="es")
                            nc.scalar.activation(es, pss,
                                                 mybir.ActivationFunctionType.Exp,
                                                 scale=SCALE)
                            if kc >= 4 * qc:
                                nc.vector.tensor_tensor(es, es,
                                                        mask_sb[:, kc - 4 * qc, :],
                                                        op=mybir.AluOpType.mult)
                            es_tiles[kc] = es

                        def accum(kc):
                            es = es_tiles[kc]
                            nc.tensor.matmul(psc, v_sb[:, kc, :], es,
                                             start=(kc == 0), stop=(kc == nkc - 1))
                            nc.tensor.matmul(psd, ones_sb, es,
                                             start=(kc == 0), stop=(kc == nkc - 1))

                        # 2-deep software pipeline so PE never waits on ACT
                        scores(0)
                        if nkc > 1:
                            scores(1)
                        for kc in range(nkc):
                            if kc + 2 < nkc:
                                scores(kc + 2)
                            accum(kc)

                        rec = nrm_pool.tile([1, 512], F32, name="rec", tag="rec")
                        nc.vector.reciprocal(rec, psd)
                        rb = nrm_pool.tile([128, 512], F32, name="rb", tag="rb")
                        nc.gpsimd.partition_broadcast(rb, rec)
                        nc.vector.tensor_tensor(ctxT[:, h, qc * 512:(qc + 1) * 512],
                                                psc, rb, op=mybir.AluOpType.mult)

            # ---------------- phase 3: fc (row-sharded partial) ----------------
            with tc.tile_pool(name="fc_sb", bufs=1) as fc_sb, \
                 tc.tile_pool(name="out_sb", bufs=3) as out_sb, \
                 tc.tile_pool(name="ps_fc", bufs=3, space="PSUM") as ps_fc:
                fcw_sb = fc_sb.tile([128, HG, D], F16)
                nc.sync.dma_start(out=fcw_sb, in_=fcw.rearrange("(h p) n -> p h n", p=128))
                for sqt in range(S // 128):
                    ob = out_sb.tile([128, D], F32, name="ob", tag="ob")
                    for nf in range(4):
                        psf = ps_fc.tile([128, 512], F32, name="psf", tag="psf")
                        for h in range(HG):
                            nc.tensor.matmul(psf,
                                             ctxT[:, h, sqt * 128:(sqt + 1) * 128],
                                             fcw_sb[:, h, nf * 512:(nf + 1) * 512],
                                             start=(h == 0), stop=(h == HG - 1))
                        nc.vector.tensor_copy(ob[:, nf * 512:(nf + 1) * 512], psf)
                    nc.sync.dma_start(out=out[sqt * 128:(sqt + 1) * 128, :], in_=ob)

    nc.compile()
    return nc


def _get_compiled():
    global _COMPILED
    if _COMPILED is None:
        _COMPILED = _build()
    return _COMPILED


def _prep_inputs(x, w_q, w_kv, fc_w, fc_b, freqs_cos, freqs_sin):
    x = np.asarray(x, dtype=np.float32)
    w_q = np.asarray(w_q, dtype=np.float32)
    w_kv = np.asarray(w_kv, dtype=np.float32)
    fc_w = np.asarray(fc_w, dtype=np.float32)
    freqs_cos = np.asarray(freqs_cos, dtype=np.float32)
    freqs_sin = np.asarray(freqs_sin, dtype=np.float32)

    # rope pair permutation: evens then odds within each head's DK block
    perm = np.concatenate([np.arange(0, DK, 2), np.arange(1, DK, 2)])

    cosT = np.ascontiguousarray(freqs_cos.T)  # [64, S]
    sinT = np.ascontiguousarray(freqs_sin.T)

    # masks[i, t, j] = 1 if i <= j - 128*t  (diagonal tiles, t = kc - 4*qc)
    i_idx = np.arange(128)[:, None, None]
    t_idx = np.arange(4)[None, :, None]
    j_idx = np.arange(512)[None, None, :]
    masks = (i_idx <= j_idx - 128 * t_idx).astype(np.float16)
    onesc = np.ones((128, 1), dtype=np.float16)

    in_maps = []
    for c in range(8):
        b, g = divmod(c, 4)
        xT = np.ascontiguousarray(x[b].T).astype(np.float16)
        wq_g = w_q[:, g * HG * DK:(g + 1) * HG * DK].reshape(D, HG, DK)[:, :, perm]
        wq_g = np.ascontiguousarray(wq_g.reshape(D, HG * DK)).astype(np.float16)
        wk_g = np.ascontiguousarray(w_kv[:, g * DK:(g + 1) * DK][:, perm]).astype(np.float16)
        wv_g = np.ascontiguousarray(w_kv[:, HKV * DK + g * DK:HKV * DK + (g + 1) * DK]).astype(np.float16)
        fcw_g = np.ascontiguousarray(fc_w[g * HG * DK:(g + 1) * HG * DK, :]).astype(np.float16)
        in_maps.append({
            "xT": xT, "wq": wq_g, "wk": wk_g, "wv": wv_g, "fcw": fcw_g,
            "cosT": cosT, "sinT": sinT, "masks": masks, "onesc": onesc,
        })
    return in_maps


def kernel_run(trace=False, **inputs):
    nc = _get_compiled()
    in_maps = _prep_inputs(**inputs)
    res = run_bass_kernel_spmd(nc, in_maps, core_ids=list(range(8)), trace=trace)
    fc_b = np.asarray(inputs["fc_b"], dtype=np.float32)
    out = np.zeros((B, S, D), dtype=np.float32)
    for c in range(8):
        b = c // 4
        out[b] += res.results[c]["out"]
    out += fc_b[None, None, :]
    return out, res


def kernel(**inputs):
    out, _ = kernel_run(trace=False, **inputs)
    return out


# revision 4
# speedup vs baseline: 1.1998x; 1.1998x over previous
"""GQA multi-head attention (B=2, S=2048, D=2048, HQ=16, HKV=4, DK=128) with
RoPE + causal softmax + output projection, sharded over 8 NeuronCores as
(batch x kv-head-group): core c handles batch c//4, kv head c%4 (4 query
heads). w_q/w_kv column-sharded, fc row-sharded; partial fc outputs are
summed on the host (the "all-reduce").
"""

import sys

for _p in ("/opt/trn_rl_repo", "/root/.axon_site", "/root/.axon_site/_ro/trn_rl_repo"):
    if _p not in sys.path:
        sys.path.insert(0, _p)

import numpy as np

import concourse.bass as bass
import concourse.mybir as mybir
import concourse.tile as tile
from concourse import bacc
from concourse.bass_utils import run_bass_kernel_spmd

F32 = mybir.dt.float32
F16 = mybir.dt.float16

B, S, D = 2, 2048, 2048
HKV, NREP, DK = 4, 4, 128
HG = NREP  # query heads per core
KC = D // 128  # contraction chunks
SQC = S // 512  # 512-wide query column chunks
SCALE = float(1.0 / np.sqrt(DK))

_COMPILED = None


def _build():
    nc = bacc.Bacc(None, target_bir_lowering=False, debug=False)

    xT = nc.dram_tensor("xT", [D, S], F16, kind="ExternalInput")
    wq = nc.dram_tensor("wq", [D, HG * DK], F16, kind="ExternalInput")
    wk = nc.dram_tensor("wk", [D, DK], F16, kind="ExternalInput")
    wv = nc.dram_tensor("wv", [D, DK], F16, kind="ExternalInput")
    fcw = nc.dram_tensor("fcw", [HG * DK, D], F16, kind="ExternalInput")
    cosT = nc.dram_tensor("cosT", [64, S], F32, kind="ExternalInput")
    sinT = nc.dram_tensor("sinT", [64, S], F32, kind="ExternalInput")
    masks = nc.dram_tensor("masks", [128, 4, 512], F16, kind="ExternalInput")
    onesc = nc.dram_tensor("onesc", [128, 1], F16, kind="ExternalInput")
    out = nc.dram_tensor("out", [S, D], F32, kind="ExternalOutput")

    with tile.TileContext(nc) as tc:
        with tc.tile_pool(name="persist", bufs=1) as persist:
            # attention-phase residents
            qt_sb = persist.tile([128, HG, S], F16)  # Q^T, rope'd, per head
            kt_sb = persist.tile([128, S], F16)  # K^T rope'd
            v_sb = persist.tile([128, KC, DK], F16)  # V  [sk, dk] chunks
            ctxT = persist.tile([128, HG, S], F16)  # (softmax @ V)^T per head
            cos_sb = persist.tile([64, S], F32)
            sin_sb = persist.tile([64, S], F32)
            mask_sb = persist.tile([128, 4, 512], F16)
            ones_sb = persist.tile([128, 1], F16)
            fcw_sb = persist.tile([128, HG, D], F16)
            # small tensors go on the ACT HWDGE ring so they don't queue
            # behind the bulk xT loads on the SP ring
            nc.scalar.dma_start(out=cos_sb, in_=cosT[:])
            nc.scalar.dma_start(out=sin_sb, in_=sinT[:])
            nc.scalar.dma_start(out=mask_sb, in_=masks[:])
            nc.scalar.dma_start(out=ones_sb, in_=onesc[:])

            psA = tc.alloc_tile_pool(name="psA", bufs=3, space="PSUM")
            psB = tc.alloc_tile_pool(name="psB", bufs=2, space="PSUM")
            psD = tc.alloc_tile_pool(name="psD", bufs=2, space="PSUM")

            # ---------------- phase 1: projections + rope ----------------
            with tc.tile_pool(name="p1sb", bufs=1) as p1sb, \
                 tc.tile_pool(name="p1tmp", bufs=3) as p1tmp:
                xt_sb = p1sb.tile([128, KC, S], F16)
                wq_sb = p1sb.tile([128, KC, HG * DK], F16)
                wk_sb = p1sb.tile([128, KC, DK], F16)
                wv_sb = p1sb.tile([128, KC, DK], F16)
                nc.scalar.dma_start(out=wk_sb, in_=wk.rearrange("(k p) m -> p k m", p=128))
                # chunked xT load: matmuls can start as soon as chunk 0 lands
                xr = xT.rearrange("(k p) s -> p k s", p=128)
                for k in range(KC):
                    nc.sync.dma_start(out=xt_sb[:, k, :], in_=xr[:, k, :])
                nc.scalar.dma_start(out=wv_sb, in_=wv.rearrange("(k p) m -> p k m", p=128))
                nc.scalar.dma_start(out=wq_sb, in_=wq.rearrange("(k p) m -> p k m", p=128))
                nc.scalar.dma_start(out=fcw_sb, in_=fcw.rearrange("(h p) n -> p h n", p=128))

                def rope(dst, ps, qc):
                    c = cos_sb[:, qc * 512:(qc + 1) * 512]
                    s = sin_sb[:, qc * 512:(qc + 1) * 512]
                    pe, po = ps[0:64, :], ps[64:128, :]
                    t1 = p1tmp.tile([64, 512], F32, name="t1", tag="t1")
                    t2 = p1tmp.tile([64, 512], F32, name="t2", tag="t2")
                    nc.vector.tensor_tensor(t1, pe, c, op=mybir.AluOpType.mult)
                    nc.vector.tensor_tensor(t2, po, s, op=mybir.AluOpType.mult)
                    nc.vector.tensor_tensor(dst[0:64, :], t1, t2, op=mybir.AluOpType.subtract)
                    t3 = p1tmp.tile([64, 512], F32, name="t3", tag="t3")
                    t4 = p1tmp.tile([64, 512], F32, name="t4", tag="t4")
                    nc.vector.tensor_tensor(t3, pe, s, op=mybir.AluOpType.mult)
                    nc.vector.tensor_tensor(t4, po, c, op=mybir.AluOpType.mult)
                    nc.vector.tensor_tensor(dst[64:128, :], t3, t4, op=mybir.AluOpType.add)

                # K^T = wk^T @ xT  (then rope)
                for qc in range(SQC):
                    psk = psA.tile([128, 512], F32, name="psk", tag="pp")
                    for k in range(KC):
                        nc.tensor.matmul(psk, wk_sb[:, k, :],
                                         xt_sb[:, k, qc * 512:(qc + 1) * 512],
                                         start=(k == 0), stop=(k == KC - 1))
                    rope(kt_sb[:, qc * 512:(qc + 1) * 512], psk, qc)

                # V = xT^T @ wv : out [sk, dk], 4 tiles packed per psum bank
                for gq in range(4):
                    psv = psB.tile([128, 512], F32, name="psv", tag="pv")
                    for vt in range(4):
                        skt = gq * 4 + vt
                        for k in range(KC):
                            nc.tensor.matmul(psv[:, vt * 128:(vt + 1) * 128],
                                             xt_sb[:, k, skt * 128:(skt + 1) * 128],
                                             wv_sb[:, k, :],
                                             start=(k == 0), stop=(k == KC - 1))
                    nc.vector.tensor_copy(
                        v_sb[:, gq * 4:(gq + 1) * 4, :].rearrange("p a b -> p (a b)"),
                        psv)

                # Q^T = wq^T @ xT (then rope)
                for mh in range(HG):
                    for qc in range(SQC):
                        psq = psA.tile([128, 512], F32, name="psq", tag="pp")
                        for k in range(KC):
                            nc.tensor.matmul(psq, wq_sb[:, k, mh * 128:(mh + 1) * 128],
                                             xt_sb[:, k, qc * 512:(qc + 1) * 512],
                                             start=(k == 0), stop=(k == KC - 1))
                        rope(qt_sb[:, mh, qc * 512:(qc + 1) * 512], psq, qc)

            # ---------------- phase 2: attention ----------------
            with tc.tile_pool(name="es_pool", bufs=6) as es_pool, \
                 tc.tile_pool(name="nrm_pool", bufs=2) as nrm_pool:
                for h in range(HG):
                    for qc in range(SQC):
                        nkc = 4 * (qc + 1)  # causal: sk chunks 0..nkc-1
                        psc = psB.tile([128, 512], F32, name="psc", tag="pv")
                        psd = psD.tile([1, 512], F32, name="psd", tag="psd")
                        qs = qt_sb[:, h, qc * 512:(qc + 1) * 512]
                        es_tiles = [None] * nkc

                        def scores(kc):
                            pss = psA.tile([128, 512], F32, name="pss", tag="pp")
                            nc.tensor.matmul(pss, kt_sb[:, kc * 128:(kc + 1) * 128],
                                             qs, start=True, stop=True)
                            es = es_pool.tile([128, 512], F16, name="es", tag="es")
                            nc.scalar.activation(es, pss,
                                                 mybir.ActivationFunctionType.Exp,
                                                 scale=SCALE)
                            if kc >= 4 * qc:
                                nc.vector.tensor_tensor(es, es,
                                                        mask_sb[:, kc - 4 * qc, :],
                                                        op=mybir.AluOpType.mult)
                            es_tiles[kc] = es

                        def accum(kc):
                            es = es_tiles[kc]
                            nc.tensor.matmul(psc, v_sb[:, kc, :], es,
                                             start=(kc == 0), stop=(kc == nkc - 1))
                            nc.tensor.matmul(psd, ones_sb, es,
                                             start=(kc == 0), stop=(kc == nkc - 1))

                        # 2-deep software pipeline so PE never waits on ACT
                        scores(0)
                        if nkc > 1:
                            scores(1)
                        for kc in range(nkc):
                            if kc + 2 < nkc:
                                scores(kc + 2)
                            accum(kc)

                        rec = nrm_pool.tile([1, 512], F32, name="rec", tag="rec")
                        nc.vector.reciprocal_approx_fast(rec, psd)
                        rb = nrm_pool.tile([128, 512], F32, name="rb", tag="rb")
                        nc.gpsimd.partition_broadcast(rb, rec)
                        nc.vector.tensor_tensor(ctxT[:, h, qc * 512:(qc + 1) * 512],
                                                psc, rb, op=mybir.AluOpType.mult)

            # ---------------- phase 3: fc (row-sharded partial) ----------------
            with tc.tile_pool(name="out_sb", bufs=3) as out_sb:
                for sqt in range(S // 128):
                    ob = out_sb.tile([128, D], F32, name="ob", tag="ob")
                    for nf in range(4):
                        psf = psA.tile([128, 512], F32, name="psf", tag="pp")
                        for h in range(HG):
                            nc.tensor.matmul(psf,
                                             ctxT[:, h, sqt * 128:(sqt + 1) * 128],
                                             fcw_sb[:, h, nf * 512:(nf + 1) * 512],
                                             start=(h == 0), stop=(h == HG - 1))
                        nc.vector.tensor_copy(ob[:, nf * 512:(nf + 1) * 512], psf)
                    nc.sync.dma_start(out=out[sqt * 128:(sqt + 1) * 128, :], in_=ob)

            psD.release()
            psB.release()
            psA.release()

    nc.compile()
    return nc


def _get_compiled():
    global _COMPILED
    if _COMPILED is None:
        _COMPILED = _build()
    return _COMPILED


def _prep_inputs(x, w_q, w_kv, fc_w, fc_b, freqs_cos, freqs_sin):
    x = np.asarray(x, dtype=np.float32)
    w_q = np.asarray(w_q, dtype=np.float32)
    w_kv = np.asarray(w_kv, dtype=np.float32)
    fc_w = np.asarray(fc_w, dtype=np.float32)
    freqs_cos = np.asarray(freqs_cos, dtype=np.float32)
    freqs_sin = np.asarray(freqs_sin, dtype=np.float32)

    # rope pair permutation: evens then odds within each head's DK block
    perm = np.concatenate([np.arange(0, DK, 2), np.arange(1, DK, 2)])

    cosT = np.ascontiguousarray(freqs_cos.T)  # [64, S]
    sinT = np.ascontiguousarray(freqs_sin.T)

    # masks[i, t, j] = 1 if i <= j - 128*t  (diagonal tiles, t = kc - 4*qc)
    i_idx = np.arange(128)[:, None, None]
    t_idx = np.arange(4)[None, :, None]
    j_idx = np.arange(512)[None, None, :]
    masks = (i_idx <= j_idx - 128 * t_idx).astype(np.float16)
    onesc = np.ones((128, 1), dtype=np.float16)

    in_maps = []
    for c in range(8):
        b, g = divmod(c, 4)
        xT = np.ascontiguousarray(x[b].T).astype(np.float16)
        wq_g = w_q[:, g * HG * DK:(g + 1) * HG * DK].reshape(D, HG, DK)[:, :, perm]
        wq_g = np.ascontiguousarray(wq_g.reshape(D, HG * DK)).astype(np.float16)
        wk_g = np.ascontiguousarray(w_kv[:, g * DK:(g + 1) * DK][:, perm]).astype(np.float16)
        wv_g = np.ascontiguousarray(w_kv[:, HKV * DK + g * DK:HKV * DK + (g + 1) * DK]).astype(np.float16)
        fcw_g = np.ascontiguousarray(fc_w[g * HG * DK:(g + 1) * HG * DK, :]).astype(np.float16)
        in_maps.append({
            "xT": xT, "wq": wq_g, "wk": wk_g, "wv": wv_g, "fcw": fcw_g,
            "cosT": cosT, "sinT": sinT, "masks": masks, "onesc": onesc,
        })
    return in_maps


def kernel_run(trace=False, **inputs):
    nc = _get_compiled()
    in_maps = _prep_inputs(**inputs)
    res = run_bass_kernel_spmd(nc, in_maps, core_ids=list(range(8)), trace=trace)
    fc_b = np.asarray(inputs["fc_b"], dtype=np.float32)
    out = np.zeros((B, S, D), dtype=np.float32)
    for c in range(8):
        b = c // 4
        out[b] += res.results[c]["out"]
    out += fc_b[None, None, :]
    return out, res


def kernel(**inputs):
    out, _ = kernel_run(trace=False, **inputs)
    return out
